# revision 1
# baseline (speedup 1.0000x reference)
"""Trainium2 Bass kernel for nn_BatchRankingLoss (pairwise hinge ranking loss).

Math: with o = squeeze(input), l = gdt_ts, the reference computes
    loss = sum_{i,j} [|l_i-l_j| > 0.1] * max(0, 1 + y_ij (o_i - o_j)) / (B(B-1))
with y_ij = -1 if l_i < l_j else +1.  For any pair with l_i > l_j + 0.1 both
orderings contribute the same hinge max(0, 1 + o_i - o_j), so
    loss * N = 2 * sum_{(i,j): fl(l_i - l_j) > 0.1} relu(1 + o_i - o_j).

Strategy (sorted prefix): sort rows by l.  The mask set for row i becomes an
exact prefix j < c_i in sorted column order (c_i computed exactly on host with
the fp32 predicate).  The device then only sums relu((1 + o_i) - o_j) over
per-row prefixes:
  - bulk: per 128-row block, the columns below the block-min prefix need no
    masking at all -> one scalar_tensor_tensor per block computing
    sum(max(R + s_i, 0)) with a per-partition fused accumulator.
  - band: the remaining <=W columns per block are masked exactly with an
    iota < (c_i - C_b) comparison.
Cores split the bulk columns by j % 8 (uniform shapes -> one SPMD program,
per-core content differs only in data) and the band blocks round-robin.
"""

import sys
import numpy as np

sys.path.insert(0, "/opt/trn_rl_repo")

import ml_dtypes  # noqa: E402
import concourse.bass as bass  # noqa: E402
import concourse.mybir as mybir  # noqa: E402
import concourse.tile as tile  # noqa: E402
from concourse.bass_utils import run_bass_kernel_spmd  # noqa: E402


def _split_multi_waits(nc, limit=1):
    """This container's walrus build rejects instructions carrying more than
    one sync-wait ("Too many sync wait commands").  Hoist all-but-`limit`
    waits of every instruction into dedicated single-wait Drain instructions
    on the same engine, inserted immediately before it — semantically
    identical (the engine blocks on each wait in order)."""
    n = 0
    for fn in nc.m.functions:
        for bb in fn.blocks:
            insts = list(bb.instructions)
            out = []
            for ins in insts:
                si = ins.sync_info
                ow = list(si.on_wait) if (si is not None and si.on_wait) else []
                if len(ow) > limit:
                    for w in ow[:-limit]:
                        n += 1
                        d = mybir.InstDrain(name=f"antwaitsplit_{n}", ins=[], outs=[])
                        d.engine = ins.engine
                        d.sync_info = mybir.SyncInfo(on_wait=[w], on_update=[])
                        out.append(d)
                    ins.sync_info = mybir.SyncInfo(
                        on_wait=ow[-limit:], on_update=list(si.on_update))
                out.append(ins)
            if len(out) != len(insts):
                bb.instructions = out
    return n

GAP = np.float32(1.0)
THRESHOLD = np.float32(0.1)
NCORES = 8
P = 128
PAD_NEG = np.float32(30000.0)  # padding o value; -PAD_NEG stays masked/relu'd to 0

BF = mybir.dt.bfloat16
F32 = mybir.dt.float32

_cache = {}


def _exact_prefix_counts(ls):
    """c_i = #{j : fl(ls_i - ls_j) > 0.1} with exact fp32 semantics.

    ls ascending => the predicate is non-increasing in j, so c_i is the first
    false index, found by vectorized binary search with the fp32 predicate.
    """
    B = ls.shape[0]
    lo = np.zeros(B, np.int64)
    hi = np.full(B, B, np.int64)
    for _ in range(14):
        act = lo < hi
        mid = (lo + hi) // 2
        midc = np.minimum(mid, B - 1)
        pred = (ls - ls[midc]).astype(np.float32) > THRESHOLD
        lo = np.where(act & pred, mid + 1, lo)
        hi = np.where(act & ~pred, mid, hi)
    return lo


def _build_program(Fb, W, NB, reps=1):
    """One SPMD program; all shapes derive from global (core-independent)
    values Fb[b] (bulk strided prefix lengths), W (band width), NB (#blocks)."""
    NR = NB * P // NCORES          # per-core strided column count (1024)
    BPC = NB // NCORES             # band blocks per core (8)
    # f32-unit column layout of the single input pack
    off_r = 0                      # R: NR bf16 = NR/2 f32 cols
    off_rb = off_r + NR // 2       # RB: BPC*W bf16 = BPC*W/2 f32 cols
    off_iota = off_rb + BPC * W // 2
    off_scal = off_iota + W
    off_sband = off_scal + NB
    off_eed = off_sband + BPC
    tot = off_eed + BPC
    NACC = NB + BPC

    nc = bass.Bass()
    pack = nc.declare_dram_parameter("pack", [P, tot], F32, isOutput=False)
    out = nc.declare_dram_parameter("out", [P, NACC], F32, isOutput=True)

    with tile.TileContext(nc) as tc:
        with tc.tile_pool(name="sbuf", bufs=1) as pool:
            ta = pool.tile([P, tot], F32)
            nsc = tot - off_scal
            scal_sb = pool.tile([P, nsc], F32)
            zeros = pool.tile([P, NR], BF)
            scratch = pool.tile([P, NR], BF)
            mt = pool.tile([P, W], BF)
            wt = pool.tile([P, W], BF)
            st = pool.tile([P, W], BF)
            acc = pool.tile([P, NACC], F32)

            nc.vector.memset(acc[:], 0.0)
            nc.vector.memset(zeros[:], 0.0)
            nc.sync.dma_start(ta[:], pack[:])
            # Single funnel op: the only instruction carrying the input-DMA
            # wait (walrus allows one sync-wait per compute instruction); all
            # later ops read scalars from scal_sb so they depend on this copy,
            # and the engine's observed DMA clock covers their ta reads.
            nc.vector.tensor_copy(scal_sb[:], ta[:, off_scal:tot])
            sc = off_scal

            r_bf = ta[:, off_r:off_rb].bitcast(BF)        # [P, NR]
            rb_bf = ta[:, off_rb:off_iota].bitcast(BF)    # [P, BPC*W]

            for _rep in range(reps):
                # bulk: acc[:, b] = sum_f max(R[:, :Fb] + scal_b, 0)
                for b in range(NB):
                    f = Fb[b]
                    if f == 0:
                        continue
                    nc.vector.scalar_tensor_tensor(
                        scratch[:, :f], r_bf[:, :f],
                        scal_sb[:, off_scal - sc + b:off_scal - sc + b + 1], zeros[:, :f],
                        mybir.AluOpType.add, mybir.AluOpType.max,
                        accum_out=acc[:, b:b + 1])

                # band: acc[:, NB+t] = sum relu((RB_t + sband_t) * [iota < eed_t])
                for t in range(BPC):
                    nc.vector.tensor_scalar(
                        mt[:], ta[:, off_iota:off_iota + W],
                        scal_sb[:, off_eed - sc + t:off_eed - sc + t + 1], None,
                        mybir.AluOpType.is_lt)
                    nc.vector.scalar_tensor_tensor(
                        wt[:], rb_bf[:, t * W:(t + 1) * W],
                        scal_sb[:, off_sband - sc + t:off_sband - sc + t + 1], mt[:],
                        mybir.AluOpType.add, mybir.AluOpType.mult)
                    nc.vector.tensor_scalar(
                        st[:], wt[:], 0.0, None,
                        mybir.AluOpType.max, mybir.AluOpType.add,
                        accum_out=acc[:, NB + t:NB + t + 1])

            # Issue from DVE: its observed clock already covers both the input
            # DMA (funnel copy wait) and the accum writes, so this instruction
            # needs at most one sync-wait.
            nc.sync.dma_start(out[:], acc[:])
    _split_multi_waits(nc)
    return nc, tot, NACC


def _prepare(input, gdt_ts):
    o = np.asarray(input, np.float32).reshape(-1)
    l = np.asarray(gdt_ts, np.float32).reshape(-1)
    B = o.shape[0]
    assert B % (P * NCORES) == 0, f"B={B} must be a multiple of {P * NCORES}"
    NB = B // P

    perm = np.argsort(l, kind="stable")
    ls = l[perm]
    osr = o[perm]
    c = _exact_prefix_counts(ls)

    cmin = c.reshape(NB, P).min(axis=1)
    cmax = c.reshape(NB, P).max(axis=1)
    Cb = (cmin // 8) * 8
    Fb = (Cb // 8).astype(np.int64)
    W = int(max(8, -(-int((cmax - Cb).max()) // 8) * 8))

    neg_os_pad = np.concatenate([-osr, np.full(W, -PAD_NEG, np.float32)])
    scal = GAP + osr.reshape(NB, P).T                    # [P, NB]
    e_all = (c - np.repeat(Cb, P)).astype(np.float32).reshape(NB, P).T  # [P, NB]
    iota = np.broadcast_to(np.arange(W, dtype=np.float32), (P, W))

    packs = []
    for k in range(NCORES):
        Rk = ml_dtypes.bfloat16(neg_os_pad[k:B:NCORES])            # [NB*P/8]
        bk = list(range(k, NB, NCORES))
        RBk = np.concatenate([ml_dtypes.bfloat16(neg_os_pad[Cb[b]:Cb[b] + W])
                              for b in bk])                        # [BPC*W]
        sband = scal[:, bk]                                        # [P, BPC]
        eed = e_all[:, bk]                                         # [P, BPC]
        pk = np.concatenate([
            np.broadcast_to(Rk.view(np.uint16), (P, Rk.shape[0])).view(np.float32).reshape(P, -1),
            np.broadcast_to(RBk.view(np.uint16), (P, RBk.shape[0])).view(np.float32).reshape(P, -1),
            iota, scal, sband, eed], axis=1).astype(np.float32)
        packs.append(np.ascontiguousarray(pk))
    return packs, Fb, W, NB, B


def kernel(input, gdt_ts):
    packs, Fb, W, NB, B = _prepare(input, gdt_ts)
    key = (tuple(Fb.tolist()), W, NB)
    if key not in _cache:
        _cache[key] = _build_program(Fb, W, NB)
    nc, tot, NACC = _cache[key]
    assert packs[0].shape == (P, tot)

    in_maps = [{"pack": packs[k]} for k in range(NCORES)]
    res = run_bass_kernel_spmd(nc, in_maps, list(range(NCORES)))
    total = np.float64(0.0)
    for k in range(NCORES):
        total += res.results[k]["out"].astype(np.float64).sum()
    loss = np.float32(2.0 * total / (B * (B - 1)))
    return loss.reshape(1)


if __name__ == "__main__":
    rng = np.random.default_rng(0)
    B = 8192
    inp = rng.standard_normal((B, 1)).astype(np.float32)
    gdt = rng.random(B, dtype=np.float32)
    print(kernel(input=inp, gdt_ts=gdt))



# revision 2
# speedup vs baseline: 6753.5494x; 6753.5494x over previous
"""Trainium2 Bass kernel v3 for nn_BatchRankingLoss (pairwise hinge ranking loss).

Sorted-prefix decomposition (see baseline docstring) with a 3-lane engine split:
  - DVE-self lane: scalar_tensor_tensor + fused accumulator (ZERO_ACCUMULATE),
    ops chained through a shared scratch tile (the WAW chain also serializes the
    accumulator read-outs, which is required for correctness).
  - ACT lane: relu activation with per-partition bias + fused accumulator;
    streams correctly with distinct outputs.
  - PE-feed lane: DVE tensor_scalar (no accum -> 4x perf mode) writes relu
    values to distinct scratch regions; the Tensor engine reduces each region
    over the partition axis with a ones-vector matmul, accumulating everything
    into a single PSUM row. Host sums the row.
Band masks are baked into per-partition band data (masked-out -> -30000).
Input DMA is split: [scalars | R] first, band data second; band ops run last.
"""

import sys
import numpy as np

sys.path.insert(0, "/opt/trn_rl_repo")

import ml_dtypes  # noqa: E402
import concourse.bass as bass  # noqa: E402
import concourse.mybir as mybir  # noqa: E402
import concourse.tile as tile  # noqa: E402
from concourse.bass_utils import run_bass_kernel_spmd  # noqa: E402


def _split_multi_waits(nc, limit=1):
    """walrus rejects instructions with more than one sync-wait: hoist excess
    waits into single-wait Drain instructions on the same engine."""
    n = 0
    for fn in nc.m.functions:
        for bb in fn.blocks:
            insts = list(bb.instructions)
            out = []
            for ins in insts:
                si = ins.sync_info
                ow = list(si.on_wait) if (si is not None and si.on_wait) else []
                if len(ow) > limit:
                    for w in ow[:-limit]:
                        n += 1
                        d = mybir.InstDrain(name=f"antwaitsplit_{n}", ins=[], outs=[])
                        d.engine = ins.engine
                        d.sync_info = mybir.SyncInfo(on_wait=[w], on_update=[])
                        out.append(d)
                    ins.sync_info = mybir.SyncInfo(
                        on_wait=ow[-limit:], on_update=list(si.on_update))
                out.append(ins)
            if len(out) != len(insts):
                bb.instructions = out
    return n




def _strip_redundant_ldweights(nc):
    """All matmuls use the same resident ones-vector weights: drop every
    InstLdweights after the first, moving any sync-waits onto single-wait
    Drain instructions in its place (same engine, semantically identical)."""
    n = 0
    for fn in nc.m.functions:
        for bb in fn.blocks:
            insts = list(bb.instructions)
            out = []
            seen = False
            changed = False
            for ins in insts:
                if type(ins).__name__ == "InstLdweights":
                    if not seen:
                        seen = True
                        out.append(ins)
                        continue
                    changed = True
                    si = ins.sync_info
                    ow = list(si.on_wait) if (si is not None and si.on_wait) else []
                    ou = list(si.on_update) if (si is not None and si.on_update) else []
                    for w in ow:
                        n += 1
                        d = mybir.InstDrain(name=f"antldw_{n}", ins=[], outs=[])
                        d.engine = ins.engine
                        d.sync_info = mybir.SyncInfo(on_wait=[w], on_update=[])
                        out.append(d)
                    if ou:
                        n += 1
                        d = mybir.InstDrain(name=f"antldwu_{n}", ins=[], outs=[])
                        d.engine = ins.engine
                        d.sync_info = mybir.SyncInfo(on_wait=[], on_update=ou)
                        out.append(d)
                    continue
                out.append(ins)
            if changed:
                bb.instructions = out
    return n


GAP = np.float32(1.0)
THRESHOLD = np.float32(0.1)
NCORES = 8
P = 128
ALIGN = 16                      # Cb alignment -> Fb even
PAD_NEG = np.float32(30000.0)
MMCH = 512                      # PE moving-tensor chunk

BF = mybir.dt.bfloat16
F32 = mybir.dt.float32
AL = mybir.AluOpType

_cache = {}


def _exact_prefix_counts(ls):
    B = ls.shape[0]
    lo = np.zeros(B, np.int64)
    hi = np.full(B, B, np.int64)
    for _ in range(14):
        act = lo < hi
        mid = (lo + hi) // 2
        midc = np.minimum(mid, B - 1)
        pred = (ls - ls[midc]).astype(np.float32) > THRESHOLD
        lo = np.where(act & pred, mid + 1, lo)
        hi = np.where(act & ~pred, mid, hi)
    return lo


# measured per-op lane costs (ns), f = free-dim columns
def _cost_dve_self(f):
    return 230.0 + 1.10 * f

def _cost_act(f):
    return 360.0 + 0.92 * f

def _cost_feed_dve(f):
    return 215.0 + 0.262 * f

def _cost_feed_act(f):
    return 280.0 + 0.47 * f

def _cost_feed_pe(f):
    full, rem = divmod(f, MMCH)
    return 300.0 * full + (80.0 + 0.58 * rem if rem else 0.0)


def _assign_lanes(Fb, W, BPC):
    """Greedy makespan assignment of items to lanes {self, act, feed}."""
    items = [("bulk", b, int(Fb[b])) for b in range(len(Fb)) if Fb[b] > 0]
    items += [("band", t, W) for t in range(BPC)]
    items.sort(key=lambda it: -it[2])
    # fixed lane tails: DVE does the PSUM-row copy at its end (~600ns)
    load = {"dve": 600.0, "act": 0.0, "pe": 0.0}
    lanes = {"self": [], "act": [], "feed": []}
    for it in items:
        f = it[2]
        cand = {
            "self": max(load["dve"] + _cost_dve_self(f), load["act"], load["pe"]),
            "act": max(load["dve"], load["act"] + _cost_act(f), load["pe"]),
            "feed": max(load["dve"] + _cost_feed_dve(f), load["act"],
                        load["pe"] + _cost_feed_pe(f)),
        }
        pick = min(cand, key=lambda k: cand[k])
        lanes[pick].append(it)
        if pick == "self":
            load["dve"] += _cost_dve_self(f)
        elif pick == "act":
            load["act"] += _cost_act(f)
        else:
            load["dve"] += _cost_feed_dve(f)
            load["pe"] += _cost_feed_pe(f)
    # band items (need DMA chunk 2) go last within each lane; feed ops run
    # smallest-first so compute starts before the full R prefix lands
    for k in lanes:
        lanes[k].sort(key=lambda it: (it[0] == "band", -it[2]))
    lanes["feed"].sort(key=lambda it: (it[0] == "band", it[2]))
    return lanes, load


def _build_program(Fb, W, NB):
    NR = NB * P // NCORES
    BPC = NB // NCORES
    off_scal = 0
    off_sband = off_scal + NB
    off_r = off_sband + BPC
    c1 = off_r + NR // 2
    off_rb = c1
    tot = off_rb + BPC * W // 2

    lanes, load = _assign_lanes(Fb, W, BPC)
    n_self = len(lanes["self"])
    n_act = len(lanes["act"])
    n_feed = len(lanes["feed"])

    nc = bass.Bass()
    pack = nc.declare_dram_parameter("pack", [P, tot], F32, isOutput=False)
    nacc = n_self + n_act
    out = nc.declare_dram_parameter("out", [P, max(nacc, 1)], F32, isOutput=True)
    outrow = nc.declare_dram_parameter("outrow", [1, MMCH], F32, isOutput=True)

    max_self = max([it[2] for it in lanes["self"]], default=2)
    feed_cols = sum(it[2] for it in lanes["feed"])
    act_cols = sum(it[2] for it in lanes["act"])

    with tile.TileContext(nc) as tc:
        with tc.tile_pool(name="sbuf", bufs=1) as pool, \
             tc.psum_pool(name="ps", bufs=1) as pp:
            ta = pool.tile([P, tot], F32)
            accv = pool.tile([P, max(n_self, 1)], F32)
            acca = pool.tile([P, max(n_act, 1)], F32)
            zeros = pool.tile([P, max_self], BF)
            sscr = pool.tile([P, max_self], BF)          # shared DVE-self out
            fscr = pool.tile([P, max(feed_cols, 2)], BF)  # distinct feed outs
            ascr = pool.tile([P, max(act_cols, 2)], BF)   # distinct ACT outs
            ones = pool.tile([P, 2], BF)
            rowbuf = pool.tile([1, MMCH], F32)
            psum = pp.tile([P, MMCH], F32)

            nc.vector.memset(zeros[:], 0.0)
            nc.gpsimd.memset(ones[:], 1.0)

            # 3-piece input DMA on one ring: a small head piece lets the
            # (ascending-size) feed ops start ~2us before the full R lands
            p0 = off_r + min(96, NR // 2)
            nc.sync.dma_start(ta[:, :p0], pack[:, :p0])
            nc.sync.dma_start(ta[:, p0:c1], pack[:, p0:c1])
            nc.gpsimd.dma_start(ta[:, c1:], pack[:, c1:])

            r_bf = ta[:, off_r:c1].bitcast(BF)
            rb_bf = ta[:, off_rb:tot].bitcast(BF)

            def srcs(it):
                kind, idx, f = it
                if kind == "bulk":
                    return r_bf[:, :f], ta[:, off_scal + idx:off_scal + idx + 1]
                return (rb_bf[:, idx * W:(idx + 1) * W],
                        ta[:, off_sband + idx:off_sband + idx + 1])

            # --- DVE feed ops first (PE consumes them) ---
            foff = [0]
            feed_regions = []
            for it in lanes["feed"]:
                s, sc = srcs(it)
                f = it[2]
                dst = fscr[:, foff[0]:foff[0] + f]
                foff[0] += f
                nc.vector.tensor_scalar(dst, s, sc, 0.0, AL.add, AL.max)
                feed_regions.append((dst, f))

            # --- PE: ones-matmul partition-reduce, accumulate into psum row ---
            first = True
            nmm_total = sum(-(-f // MMCH) for _, f in feed_regions)
            mm = 0
            for dst, f in feed_regions:
                for k in range(0, f, MMCH):
                    w = min(MMCH, f - k)
                    mm += 1
                    nc.tensor.matmul(psum[0:1, :w], ones[:, 0:1], dst[:, k:k + w],
                                     start=first, stop=(mm == nmm_total),
                                     skip_group_check=not first)
                    first = False

            # --- DVE self lane: stt chain through shared scratch ---
            for i, it in enumerate(lanes["self"]):
                s, sc = srcs(it)
                f = it[2]
                nc.vector.scalar_tensor_tensor(
                    sscr[:, :f], s, sc, zeros[:, :f], AL.add, AL.max,
                    accum_out=accv[:, i:i + 1])

            # --- ACT lane ---
            aoff = [0]
            for i, it in enumerate(lanes["act"]):
                s, sc = srcs(it)
                f = it[2]
                dst = ascr[:, aoff[0]:aoff[0] + f]
                aoff[0] += f
                nc.scalar.activation(dst, s, mybir.ActivationFunctionType.Relu,
                                     bias=sc, scale=1.0,
                                     accum_out=acca[:, i:i + 1])

            # output paths: DVE copies the PSUM row at its lane end (cheap);
            # sync ships both acc tiles as soon as their lanes complete;
            # gpsimd ships the row.
            if n_feed:
                nc.vector.tensor_copy(rowbuf[:], psum[0:1, :])
            else:
                nc.gpsimd.memset(rowbuf[:], 0.0)
            nc.gpsimd.dma_start(outrow[:], rowbuf[:])
            if n_self:
                nc.sync.dma_start(out[:, :n_self], accv[:, :n_self])
            if n_act:
                nc.sync.dma_start(out[:, n_self:], acca[:, :n_act])
    _strip_redundant_ldweights(nc)
    _split_multi_waits(nc)
    return nc, tot, nacc


def _prepare(input, gdt_ts):
    o = np.asarray(input, np.float32).reshape(-1)
    l = np.asarray(gdt_ts, np.float32).reshape(-1)
    B = o.shape[0]
    assert B % (P * NCORES) == 0, f"B={B} must be a multiple of {P * NCORES}"
    NB = B // P
    BPC = NB // NCORES

    perm = np.argsort(l, kind="stable")
    ls = l[perm]
    osr = o[perm]
    c = _exact_prefix_counts(ls)

    cmin = c.reshape(NB, P).min(axis=1)
    cmax = c.reshape(NB, P).max(axis=1)
    Cb = (cmin // ALIGN) * ALIGN
    Fb = (Cb // NCORES).astype(np.int64)
    W = int(max(8, -(-int((cmax - Cb).max()) // 8) * 8))

    neg_os = ml_dtypes.bfloat16(-osr)
    neg_os_pad = np.concatenate([neg_os, ml_dtypes.bfloat16(
        np.full(W + ALIGN, -PAD_NEG, np.float32))])
    scal = (GAP + osr).reshape(NB, P).T.astype(np.float32)      # [P, NB]

    t = np.arange(W)
    colidx = Cb[:, None] + t[None, :]                           # [NB, W]
    vals = neg_os_pad[colidx]                                   # [NB, W]
    valid = colidx[:, None, :] < c.reshape(NB, P)[:, :, None]   # [NB, P, W]
    band = np.where(valid, vals[:, None, :],
                    ml_dtypes.bfloat16(-PAD_NEG))               # [NB, P, W]

    packs = []
    for k in range(NCORES):
        Rk = neg_os[k:B:NCORES]
        bk = list(range(k, NB, NCORES))
        RBk = band[bk].transpose(1, 0, 2).reshape(P, BPC * W)
        sband = scal[:, bk]
        pk = np.concatenate([
            scal, sband,
            np.ascontiguousarray(np.broadcast_to(
                Rk.view(np.uint16), (P, Rk.shape[0]))).view(np.float32),
            RBk.view(np.uint16).reshape(P, -1).view(np.float32),
        ], axis=1).astype(np.float32)
        packs.append(np.ascontiguousarray(pk))
    return packs, Fb, W, NB, B


def kernel(input, gdt_ts):
    packs, Fb, W, NB, B = _prepare(input, gdt_ts)
    key = (tuple(Fb.tolist()), W, NB)
    if key not in _cache:
        _cache[key] = _build_program(Fb, W, NB)
    nc, tot, nacc = _cache[key]
    assert packs[0].shape == (P, tot), (packs[0].shape, tot)

    in_maps = [{"pack": packs[k]} for k in range(NCORES)]
    res = run_bass_kernel_spmd(nc, in_maps, list(range(NCORES)))
    total = np.float64(0.0)
    for k in range(NCORES):
        total += res.results[k]["out"][:, :nacc].astype(np.float64).sum()
        total += res.results[k]["outrow"].astype(np.float64).sum()
    loss = np.float32(2.0 * total / (B * (B - 1)))
    return loss.reshape(1)


if __name__ == "__main__":
    rng = np.random.default_rng(0)
    B = 8192
    inp = rng.standard_normal((B, 1)).astype(np.float32)
    gdt = rng.random(B, dtype=np.float32)
    print(kernel(input=inp, gdt_ts=gdt))
